# revision 3
# baseline (speedup 1.0000x reference)
"""Trainium2 Bass kernel for nn_GatedGN (gated graph network with GRU message passing).

Strategy (pure data parallel, 8 NeuronCores):
  - Shard the N=2048 graphs as 256 graphs per core; replicate all weights.
  - Everything on-chip (SBUF-resident states), feature-major ("transposed") layout:
    tensors stored as [feature(=partition), row(=free)] so GRU gate biases become
    per-partition ACT bias vectors and H=128 exactly fills the partition dim.
  - Edge GRU input [h_i, h_j] is never materialized: the i/j broadcasts are expressed
    directly as step-0 access patterns on the matmul moving operand.
  - Node GRU edge-gather e[n,i,i+1] is a strided matmul rhs + partial strided PSUM
    accumulation (i=7 columns receive only the towers/hh/bias contributions = zeros edge).
  - bf16 for edge/node GRU weights/states/gates (DVE 2x mode), fp32 PSUM accumulation,
    fp32 for the tiny global GRU + output MLP.
"""

import numpy as np
import ml_dtypes
from contextlib import ExitStack

import concourse.bass as bass
import concourse.mybir as mybir
import concourse.tile as tile
from concourse import bacc
from concourse.bass_utils import run_bass_kernel_spmd

F32 = mybir.dt.float32
BF16 = mybir.dt.bfloat16
AF = mybir.ActivationFunctionType
ALU = mybir.AluOpType

N_CORES = 8
N, K, NIN, H, L = 2048, 8, 14, 128, 2
NLOC = N // N_CORES            # graphs per core = 256
E_COLS = NLOC * K * K          # 16384 edge-state columns per core
M_COLS = NLOC * K              # 2048 node columns per core
G_COLS = NLOC                  # 256 graph columns per core
CH = 512                       # chunk width (one PSUM bank of fp32)
NCH_E = E_COLS // CH           # 32
NCH_M = M_COLS // CH           # 4

bf16 = ml_dtypes.bfloat16


def _gru_chunk(nc, ps, wk, w, dt, gi_mms, hh_w, hh_rhs, bias, state_ap, zs):
    """One GRU cell over a column chunk, transposed layout.

    gi_mms: list of (lhsT_384, rhs_ap, out_view_or_None, full_coverage) input-side matmuls.
    hh_w:   lhsT [*, 384] applied to hh_rhs (previous state), or None when zs.
    bias:   [128, 4] AP slice, columns (r, -z, inn, hn).
    state_ap: [128, w] state chunk, updated in place.
    zs:     previous state is all-zero (first message-passing step).
    """
    Pr = ps.tile([128, w], F32, tag="pr")
    Pz = ps.tile([128, w], F32, tag="pz")
    Pi = ps.tile([128, w], F32, tag="pi")
    n_hh = 0 if zs else 1
    for g, P in ((0, Pr), (1, Pz), (2, Pi)):
        total = len(gi_mms) + (n_hh if g < 2 else 0)
        idx = 0
        for lw, rhs, out_view, full in gi_mms:
            out = P if out_view is None else out_view(P)
            nc.tensor.matmul(
                out, lw[:, g * 128:(g + 1) * 128], rhs,
                start=(idx == 0), stop=(idx == total - 1),
                skip_group_check=not full,
            )
            idx += 1
        if not zs and g < 2:
            nc.tensor.matmul(
                P, hh_w[:, g * 128:(g + 1) * 128], hh_rhs,
                start=False, stop=True, skip_group_check=True,
            )
    if not zs:
        Ph = ps.tile([128, w], F32, tag="ph")
        nc.tensor.matmul(Ph, hh_w[:, 256:384], hh_rhs, start=True, stop=True)

    r = wk.tile([128, w], dt, tag="r")
    nc.scalar.activation(r, Pr, AF.Sigmoid, bias=bias[:, 0:1])
    zp = wk.tile([128, w], dt, tag="zp")  # zp = 1 - z = sigmoid(-(Pz + b_z))
    nc.scalar.activation(zp, Pz, AF.Sigmoid, bias=bias[:, 1:2], scale=-1.0)
    t = wk.tile([128, w], dt, tag="t")
    if zs:
        nc.vector.tensor_scalar_mul(t, r, bias[:, 3:4])  # t = r * b_hn
    else:
        hnb = wk.tile([128, w], dt, tag="hnb")
        nc.scalar.activation(hnb, Ph, AF.Identity, bias=bias[:, 3:4])
        nc.vector.tensor_mul(t, r, hnb)
    u = wk.tile([128, w], dt, tag="u")
    nc.vector.tensor_add(u, t, Pi)
    nt = wk.tile([128, w], dt, tag="nt")
    nc.scalar.activation(nt, u, AF.Tanh, bias=bias[:, 2:3])
    if zs:
        nc.vector.tensor_mul(state_ap, zp, nt)          # h' = (1-z) * n
    else:
        d = wk.tile([128, w], dt, tag="d")
        nc.vector.tensor_sub(d, nt, state_ap)           # d = n - h
        q = wk.tile([128, w], dt, tag="q")
        nc.vector.tensor_mul(q, zp, d)                  # q = (1-z) * d
        nc.vector.tensor_add(state_ap, state_ap, q)     # h' = h + q


def build(k=3, reps=1, loop_reps=1):
    nc = bacc.Bacc("TRN2", target_bir_lowering=False, debug=False,
                   num_devices=N_CORES)
    d_towersT = nc.dram_tensor("towersT", [NIN, M_COLS], BF16, kind="ExternalInput").ap()
    d_Wm = nc.dram_tensor("Wm", [128, 5 * 384], BF16, kind="ExternalInput").ap()
    d_Wua = nc.dram_tensor("Wua", [NIN, 384], BF16, kind="ExternalInput").ap()
    d_Wub = nc.dram_tensor("Wub", [128, 4 * 384], BF16, kind="ExternalInput").ap()
    d_Wg = nc.dram_tensor("Wg", [128, 5 * 384], F32, kind="ExternalInput").ap()
    d_EnT = nc.dram_tensor("EnT", [NIN, 128], BF16, kind="ExternalInput").ap()
    d_biases = nc.dram_tensor("biases", [128, 28], F32, kind="ExternalInput").ap()
    d_O12T = nc.dram_tensor("O12T", [128, 256], F32, kind="ExternalInput").ap()
    d_w3T = nc.dram_tensor("w3T", [128, 1], F32, kind="ExternalInput").ap()
    d_out = nc.dram_tensor("out", [G_COLS], F32, kind="ExternalOutput").ap()

    with tile.TileContext(nc) as tc, ExitStack() as ctx:
        pp = ctx.enter_context(tc.tile_pool(name="persist", bufs=1))
        ps = ctx.enter_context(tc.tile_pool(name="psum", bufs=2, space="PSUM"))
        wk = ctx.enter_context(tc.tile_pool(name="work", bufs=3))

        towersT = pp.tile([NIN, M_COLS], BF16)
        nc.sync.dma_start(out=towersT, in_=d_towersT)
        Wm = pp.tile([128, 5 * 384], BF16)
        nc.sync.dma_start(out=Wm, in_=d_Wm)
        Wua = pp.tile([NIN, 384], BF16)
        nc.sync.dma_start(out=Wua, in_=d_Wua)
        Wub = pp.tile([128, 4 * 384], BF16)
        nc.sync.dma_start(out=Wub, in_=d_Wub)
        Wg = pp.tile([128, 5 * 384], F32)
        nc.sync.dma_start(out=Wg, in_=d_Wg)
        EnT = pp.tile([NIN, 128], BF16)
        nc.sync.dma_start(out=EnT, in_=d_EnT)
        B = pp.tile([128, 28], F32)
        nc.sync.dma_start(out=B, in_=d_biases)
        O12T = pp.tile([128, 256], F32)
        nc.sync.dma_start(out=O12T, in_=d_O12T)
        w3T = pp.tile([128, 1], F32)
        nc.sync.dma_start(out=w3T, in_=d_w3T)

        e0 = pp.tile([128, E_COLS], BF16)
        e1 = pp.tile([128, E_COLS], BF16)
        hs0 = pp.tile([128, M_COLS], BF16)
        hs1 = pp.tile([128, M_COLS], BF16)
        emb = pp.tile([128, M_COLS], BF16)
        g0 = pp.tile([128, G_COLS], F32)
        g1 = pp.tile([128, G_COLS], F32)
        hsum = pp.tile([128, G_COLS], F32)
        esum = pp.tile([128, G_COLS], F32)

        # weight block views [*, 384]
        m_a, m_b, m_h0, m_i1, m_h1 = (Wm[:, i * 384:(i + 1) * 384] for i in range(5))
        u_b, u_h0, u_i1, u_h1 = (Wub[:, i * 384:(i + 1) * 384] for i in range(4))
        g_a, g_b, g_h0, g_i1, g_h1 = (Wg[:, i * 384:(i + 1) * 384] for i in range(5))

        import contextlib

        def rep_body():
            # node embedding h = En(towers)
            for c in range(NCH_M):
                cs = slice(c * CH, (c + 1) * CH)
                Pe = ps.tile([128, CH], F32, tag="pr")
                nc.tensor.matmul(Pe, EnT, towersT[:, cs], start=True, stop=True)
                nc.scalar.activation(emb[:, cs], Pe, AF.Identity, bias=B[:, 24:25])

            for step in range(k):
                zs = step == 0
                h_in = emb if zs else hs1
                # ---- edge GRU over all (i,j) pairs ----
                for c in range(NCH_E):
                    cs = slice(c * CH, (c + 1) * CH)
                    sl = h_in[:, c * 64:(c + 1) * 64]  # 8 graphs x 8 nodes
                    rhs_i = (sl.rearrange("p (g i) -> p g i", g=8)
                             .unsqueeze(3).broadcast_to([128, 8, 8, 8]))
                    rhs_j = (sl.rearrange("p (g j) -> p g j", g=8)
                             .unsqueeze(2).broadcast_to([128, 8, 8, 8]))
                    _gru_chunk(nc, ps, wk, CH, BF16,
                               [(m_a, rhs_i, None, True), (m_b, rhs_j, None, True)],
                               None if zs else m_h0, None if zs else e0[:, cs],
                               B[:, 0:4], e0[:, cs], zs)
                    _gru_chunk(nc, ps, wk, CH, BF16,
                               [(m_i1, e0[:, cs], None, True)],
                               None if zs else m_h1, None if zs else e1[:, cs],
                               B[:, 4:8], e1[:, cs], zs)
                    nc.vector.reduce_sum(
                        out=esum[:, c * 8:(c + 1) * 8],
                        in_=e1[:, cs].rearrange("p (g x) -> p g x", g=8),
                        axis=mybir.AxisListType.X)
                # ---- node GRU ----
                for c in range(NCH_M):
                    cs = slice(c * CH, (c + 1) * CH)
                    gath = (e1[:, c * 64 * 64:(c + 1) * 64 * 64]
                            .rearrange("p (g x) -> p g x", g=64)[:, :, 1:64:9])
                    gout = (lambda P: P.rearrange("p (g i) -> p g i", g=64)[:, :, 0:7])
                    _gru_chunk(nc, ps, wk, CH, BF16,
                               [(Wua, towersT[:, cs], None, True),
                                (u_b, gath, gout, False)],
                               None if zs else u_h0, None if zs else hs0[:, cs],
                               B[:, 8:12], hs0[:, cs], zs)
                    _gru_chunk(nc, ps, wk, CH, BF16,
                               [(u_i1, hs0[:, cs], None, True)],
                               None if zs else u_h1, None if zs else hs1[:, cs],
                               B[:, 12:16], hs1[:, cs], zs)
                    nc.vector.reduce_sum(
                        out=hsum[:, c * 64:(c + 1) * 64],
                        in_=hs1[:, cs].rearrange("p (g x) -> p g x", g=64),
                        axis=mybir.AxisListType.X)
                # ---- global GRU ----
                _gru_chunk(nc, ps, wk, G_COLS, F32,
                           [(g_a, hsum, None, True), (g_b, esum, None, True)],
                           None if zs else g_h0, None if zs else g0,
                           B[:, 16:20], g0, zs)
                _gru_chunk(nc, ps, wk, G_COLS, F32,
                           [(g_i1, g0, None, True)],
                           None if zs else g_h1, None if zs else g1,
                           B[:, 20:24], g1, zs)

            # ---- output MLP ----
            P1 = ps.tile([128, G_COLS], F32, tag="pr")
            nc.tensor.matmul(P1, O12T[:, 0:128], g1, start=True, stop=True)
            x1 = wk.tile([128, G_COLS], F32, tag="r")
            nc.scalar.activation(x1, P1, AF.Relu, bias=B[:, 25:26])
            P2 = ps.tile([128, G_COLS], F32, tag="pz")
            nc.tensor.matmul(P2, O12T[:, 128:256], x1, start=True, stop=True)
            x2 = wk.tile([128, G_COLS], F32, tag="zp")
            nc.scalar.activation(x2, P2, AF.Relu, bias=B[:, 26:27])
            P3 = ps.tile([1, G_COLS], F32, tag="pi")
            nc.tensor.matmul(P3, w3T, x2, start=True, stop=True)
            o = wk.tile([1, G_COLS], F32, tag="o")
            nc.scalar.activation(o, P3, AF.Sigmoid, bias=B[0:1, 27:28])
            nc.sync.dma_start(out=d_out.unsqueeze(0), in_=o)

        for rep in range(reps):
            if loop_reps > 1:
                with tc.For_i(0, loop_reps, 1):
                    rep_body()
            else:
                rep_body()

    nc.compile()
    return nc


def _pack_inputs(towers, En_W, En_b, u_params, m_params, g_params, O_params):
    """Host-side packing: returns (shared weight map, per-core towersT list, b3)."""
    towers = np.asarray(towers, np.float32)

    def gru_pack(params, skip_a):
        """Transposed weight blocks + bias columns for one 2-layer GRU."""
        (Wih0, Whh0, bih0, bhh0), (Wih1, Whh1, bih1, bhh1) = [
            tuple(np.asarray(a, np.float32) for a in layer) for layer in params
        ]
        aT = Wih0[:, :skip_a].T if skip_a else None          # [in_a, 384]
        bT = Wih0[:, skip_a:].T                              # [128, 384]
        blocks128 = [bT, Whh0.T, Wih1.T, Whh1.T]
        bias = np.zeros((128, 8), np.float32)
        for l, (bi, bh) in enumerate(((bih0, bhh0), (bih1, bhh1))):
            bias[:, 4 * l + 0] = bi[0:128] + bh[0:128]
            bias[:, 4 * l + 1] = -(bi[128:256] + bh[128:256])
            bias[:, 4 * l + 2] = bi[256:384]
            bias[:, 4 * l + 3] = bh[256:384]
        return aT, blocks128, bias

    m_aT, m_blocks, m_bias = gru_pack(m_params, 128)
    u_aT, u_blocks, u_bias = gru_pack(u_params, NIN)
    g_aT, g_blocks, g_bias = gru_pack(g_params, 128)

    Wm = np.concatenate([m_aT] + m_blocks, axis=1).astype(bf16)      # [128, 1920]
    Wua = np.ascontiguousarray(u_aT).astype(bf16)                    # [14, 384]
    Wub = np.concatenate(u_blocks, axis=1).astype(bf16)              # [128, 1536]
    Wg = np.ascontiguousarray(
        np.concatenate([g_aT] + g_blocks, axis=1)).astype(np.float32)  # [128, 1920]

    biases = np.zeros((128, 28), np.float32)
    biases[:, 0:8] = m_bias
    biases[:, 8:16] = u_bias
    biases[:, 16:24] = g_bias
    biases[:, 24] = np.asarray(En_b, np.float32)
    (O1w, O1b), (O2w, O2b), (O3w, O3b) = [
        tuple(np.asarray(a, np.float32) for a in p) for p in O_params
    ]
    biases[:, 25] = O1b
    biases[:, 26] = O2b
    biases[:, 27] = float(np.asarray(O3b).reshape(-1)[0])

    shared = {
        "Wm": Wm,
        "Wua": Wua,
        "Wub": Wub,
        "Wg": Wg,
        "EnT": np.ascontiguousarray(np.asarray(En_W, np.float32).T).astype(bf16),
        "biases": biases,
        "O12T": np.ascontiguousarray(
            np.concatenate([O1w.T, O2w.T], axis=1)).astype(np.float32),
        "w3T": np.ascontiguousarray(O3w.T).astype(np.float32),
    }
    towersT = [
        np.ascontiguousarray(
            towers[c * NLOC:(c + 1) * NLOC].transpose(2, 0, 1).reshape(NIN, M_COLS)
        ).astype(bf16)
        for c in range(N_CORES)
    ]
    return shared, towersT, float(np.asarray(O3b).reshape(-1)[0])


_CACHE = {}


def kernel(towers, k, En_W, En_b, u_params, m_params, g_params, O_params):
    k = int(k)
    shared, towersT, b3 = _pack_inputs(
        towers, En_W, En_b, u_params, m_params, g_params, O_params)
    if k not in _CACHE:
        _CACHE[k] = build(k=k, reps=1)
    nc = _CACHE[k]
    in_maps = [dict(shared, towersT=towersT[c]) for c in range(N_CORES)]
    res = run_bass_kernel_spmd(nc, in_maps, core_ids=list(range(N_CORES)))
    out = np.concatenate([res.results[c]["out"] for c in range(N_CORES)])
    return out.astype(np.float32)
